# revision 58
# baseline (speedup 1.0000x reference)
"""Trainium2 Bass kernel for the Gaussian-mixture ray autoencoder.

Math: prob[n] = sigmoid( sum_k lab_k * exp(-0.5 * (pos_n - mu_k)^T Sigma_k^{-1} (pos_n - mu_k)) )

The quadratic form is expanded into a 16-feature bilinear form:
    q'[n,k] = -0.5 q[n,k] = F[:, n] . W[:, k]
with F = per-ray monomial features (quadratic/linear/const in centered pos)
and W = per-gaussian coefficients (folding -0.5, Sigma^-1, mu, and
log|lab| into the constant term).  The sign of lab is handled by sorting
gaussians into a positive block and a negative block and accumulating the
two blocks separately.

Precision: the bilinear form cancels catastrophically (|W| up to ~1e6
while q' ~ 1), so matmul inputs need >=20 mantissa bits.  bf16 and single
float32r (~12-bit) fail; plain fp32 runs at 1/4 PE rate.  The kernel uses
a hi/lo split in float32r and three single-rate matmuls accumulated in
PSUM:   q' ~= Fhi.Whi + Fhi.Wlo + Flo.Whi      (rel err ~1.5e-4)
issued round-robin over 4 PE row groups (tile_position) so four k-tiles
stream concurrently through the 128x128 array.

Device work per core (N/8 = 1024 rays, all K gaussians): matmuls fill
PSUM banks round-robin; ScalarE Exp (multi-bank free dim) with fused
accumulator dump per label-group chunk; final subtract + sigmoid via
exp/reciprocal; DMAs in, one DMA out.
"""

import os
import sys

import numpy as np

if "/opt/trn_rl_repo" not in sys.path:
    sys.path.insert(0, "/opt/trn_rl_repo")

N = 8192
K = 4096
NCORES = 8
NLOC = N // NCORES          # rays per core
NT = NLOC // 128            # 128-ray tiles per core
TK = 512                    # gaussians per k-tile (one PSUM bank of fp32)
NGRP = 4                    # PE row groups used for concurrent matmuls

MAXCHUNK = int(os.environ.get("KERNEL_MAXCHUNK", "3"))
SCRATCH_DT = os.environ.get("KERNEL_SCRATCH", "f8")

# index pairs for the quadratic monomials p_i * p_j
_IU = [(0, 0), (1, 1), (2, 2), (3, 3),
       (0, 1), (0, 2), (0, 3), (1, 2), (1, 3), (2, 3)]

LAST_EXEC_TIME_NS = None
_GRAPH_CACHE = {}


def _round_f32r(x):
    """Exact float32r (PE reduced-precision fp32) rounding, via neuronxcc."""
    from neuronxcc.starfish.support.dtype import (
        static_cast_fp32_to_fp32r,
        static_cast_fp32r_to_fp32,
    )

    x32 = np.ascontiguousarray(x, dtype=np.float32)
    return np.asarray(
        static_cast_fp32r_to_fp32(static_cast_fp32_to_fp32r(x32)), dtype=np.float32
    )


def _host_prep(origins, directions, embeddings, chol, labels, idx):
    """float64 host-side prep of the replicated gaussian table and ray features."""
    idx = np.asarray(idx).astype(np.int64)
    mu = np.asarray(embeddings, dtype=np.float64)[idx]        # [K,4]
    L = np.asarray(chol, dtype=np.float64)[idx]               # [K,4,4]
    lab = np.asarray(labels, dtype=np.float64)[idx]           # [K]

    Sigma = np.einsum("kij,klj->kil", L, L)
    A = np.linalg.inv(Sigma)                                  # [K,4,4]

    pos = np.concatenate(
        [np.asarray(origins, np.float64), np.asarray(directions, np.float64)], axis=1
    )                                                         # [N,4]
    # center to reduce feature magnitudes (cancellation robustness)
    center = 0.5
    pos_c = pos - center
    mu_c = mu - center

    b = np.einsum("kij,kj->ki", A, mu_c)                      # [K,4]
    c = np.einsum("ki,ki->k", mu_c, b)                        # [K]

    kk = idx.shape[0]
    W = np.zeros((16, kk), dtype=np.float64)
    for r, (i, j) in enumerate(_IU):
        W[r] = -0.5 * A[:, i, j] if i == j else -A[:, i, j]
    W[10:14] = b.T
    # constant term: -0.5*c + log|lab|  (lab==0 -> force exp to 0)
    with np.errstate(divide="ignore"):
        loglab = np.where(lab == 0.0, -1e4, np.log(np.abs(np.where(lab == 0, 1.0, lab))))
    W[14] = -0.5 * c + loglab

    sgn = np.sign(lab)
    pos_mask = sgn > 0
    # order: positive-label gaussians first, then the rest
    order = np.concatenate([np.nonzero(pos_mask)[0], np.nonzero(~pos_mask)[0]])
    W = W[:, order]
    P = int(pos_mask.sum())

    # pad each group to an even count (fp32r wants even widths); the last
    # tile of each group is PARTIAL so the padding work stays ~zero
    npos = P + (P & 1)
    nneg = (kk - P) + ((kk - P) & 1)
    nkt_pos = max(1, (npos + TK - 1) // TK)
    nkt_neg = (nneg + TK - 1) // TK
    ktot = (nkt_pos + nkt_neg) * TK
    Wp = np.zeros((16, ktot), dtype=np.float64)
    Wp[14, :] = -1e4                                          # padding cols -> exp()=0
    Wp[:, : P] = W[:, :P]
    Wp[:, nkt_pos * TK : nkt_pos * TK + (kk - P)] = W[:, P:]

    F = np.zeros((16, N), dtype=np.float64)
    for r, (i, j) in enumerate(_IU):
        F[r] = pos_c[:, i] * pos_c[:, j]
    F[10:14] = pos_c.T
    F[14] = 1.0

    return Wp.astype(np.float32), F.astype(np.float32), nkt_pos, nkt_neg, npos, nneg


def _tiles_and_chunks(nkt_pos, nkt_neg, npos, nneg, t):
    """tiles: [(j, group, width)] where width<=TK (the last tile of each
    group is partial).  chunks: [(j0, ln, group, fd)] runs of <=MAXCHUNK
    tiles with consecutive PSUM banks inside one group.  The bank of tile
    j is (t*NKT+j) % 8 -- a global rotation so consecutive n-tiles never
    collide on a bank at the boundary."""
    NKT = nkt_pos + nkt_neg
    tiles = []
    for j in range(NKT):
        if j < nkt_pos:
            w = min(TK, npos - j * TK)
        else:
            w = min(TK, nneg - (j - nkt_pos) * TK)
        tiles.append((j, 0 if j < nkt_pos else 1, w))
    chunks = []
    for g, (j0g, j1g) in enumerate([(0, nkt_pos), (nkt_pos, NKT)]):
        j = j0g
        while j < j1g:
            maxc = MAXCHUNK
            if t == 0 and j == 0:
                maxc = 1        # tiny first chunk: ScalarE starts sooner
            ln = 1
            while ln < maxc and j + ln < j1g and (t * NKT + j + ln) % 8 != 0:
                ln += 1
            fd = sum(tiles[jj][2] for jj in range(j, j + ln))
            chunks.append((j, ln, g, fd))
            j += ln
    return tiles, chunks


def _build_graph_raw(key):
    """Hand-rolled (non-Tile) build: explicit semaphores, no Tile exit
    machinery (saves ~10us of drain/barrier/sem-clear tail)."""
    nkt_pos, nkt_neg, npos, nneg = key
    import concourse.bass as bass
    import concourse.mybir as mybir

    f32 = mybir.dt.float32
    f32r = mybir.dt.float32r
    Exp = mybir.ActivationFunctionType.Exp
    scratch_dt = {
        "f8": mybir.dt.float8e4, "bf16": mybir.dt.bfloat16, "f32": f32
    }[SCRATCH_DT]

    NKT = nkt_pos + nkt_neg
    KTOT = NKT * TK
    per_t = [_tiles_and_chunks(nkt_pos, nkt_neg, npos, nneg, t) for t in range(NT)]
    ncp = max(sum(1 for c in ch if c[2] == 0) for _, ch in per_t)
    ncn = max(sum(1 for c in ch if c[2] == 1) for _, ch in per_t)

    # ---- schedule precomputation (pure python) ----
    sched = []            # [{tiles:[(bank,w,grp,slot,war_need)], psem_need,
                          #   b0, fd, scr_off, acc:(which,col)}]
    tile_seq = 0
    chunk_tick = 0
    bank_last_reader = {b: 0 for b in range(8)}
    for t in range(NT):
        tiles, chunks = per_t[t]
        cpt = cnt_ = 0
        for (j0, ln, g, fd) in chunks:
            rec_tiles = []
            for j in range(j0, j0 + ln):
                w = tiles[j][2]
                bank = (t * NKT + j) % 8
                rec_tiles.append(
                    (bank, w, j % NGRP, j // NGRP, bank_last_reader[bank])
                )
                tile_seq += 1
            chunk_tick += 1
            for (bank, _, _, _, _) in rec_tiles:
                bank_last_reader[bank] = chunk_tick
            if g == 0:
                acc = ("p", t * ncp + cpt); cpt += 1
            else:
                acc = ("n", t * ncn + cnt_); cnt_ += 1
            sched.append({
                "tiles": rec_tiles, "psem_need": tile_seq,
                "b0": (t * NKT + j0) % 8, "fd": fd,
                "scr_off": t * KTOT + j0 * TK, "acc": acc, "t": t,
            })
    NCH = len(sched)

    nc = bass.Bass()
    smax = (NKT + NGRP - 1) // NGRP
    # column layout (so each DMA gating set is ONE contiguous full-BW DMA):
    #  [Fhi t0 | Flo t0 | Whi s0 | Wlo s0 | pad | Fhi t1..7 | Flo t1..7 | Whi s1+ | Wlo s1+]
    FB2 = 3 * TK                      # 1536: start of F t1..7 block
    WB2 = FB2 + 2 * (NLOC - 128)      # start of W slot1+ block
    X = WB2 + 2 * (smax - 1) * TK
    wfd = nc.declare_dram_parameter("wf", [128, X], f32r, isOutput=False)
    outd = nc.declare_dram_parameter("out", [128, NT], f32, isOutput=True)

    def fh_cols(t):
        return (0, 128) if t == 0 else (FB2 + (t - 1) * 128, 128)

    def fl_cols(t):
        return (128, 256 - 128) if t == 0 else (
            FB2 + (NLOC - 128) + (t - 1) * 128, 128)

    def wh_cols(slot, w):
        return (256 + 0, w) if slot == 0 else (WB2 + (slot - 1) * TK, w)

    def wl_cols(slot, w):
        return (256 + TK, w) if slot == 0 else (
            WB2 + (smax - 1) * TK + (slot - 1) * TK, w)

    with (
        nc.sbuf_tensor("wfsb", [128, X], f32r) as wfsb,
        nc.sbuf_tensor("accp", [128, NT * ncp], f32) as accp,
        nc.sbuf_tensor("accn", [128, max(NT * ncn, 1)], f32) as accn,
        nc.sbuf_tensor("scratch", [128, NT * KTOT], scratch_dt) as scratch,
        nc.sbuf_tensor("epil", [128, 6 * NT + 8], f32) as epil,
        nc.psum_tensor("psall", [128, 8 * TK], f32) as psall,
        nc.semaphore("dsemA") as dsemA,
        nc.semaphore("dsemA2") as dsemA2,
        nc.semaphore("dsemBW") as dsemBW,
        nc.semaphore("dsemBF") as dsemBF,
        nc.semaphore("psem") as psem,
        nc.semaphore("asem") as asem,
        nc.semaphore("vsem") as vsem,
        nc.semaphore("osem") as osem,
        nc.Block(no_gpsimd_drain=True) as block,
    ):
        spos = epil[:, 0 * NT : 1 * NT]
        sneg = epil[:, 1 * NT : 2 * NT]
        s_ = epil[:, 2 * NT : 3 * NT]
        z = epil[:, 3 * NT : 4 * NT]
        zp = epil[:, 4 * NT : 5 * NT]
        prob = epil[:, 5 * NT : 6 * NT]
        dummy = epil[:, 6 * NT : 6 * NT + 1]

        @block.sync
        def _(sync):
            # gating set A1: F(t0) + Whi(slot0); A2: Wlo(slot0) (only one
            # pass of a tile touches Wlo, so PE can start sooner)
            sync.dma_start(out=wfsb[:, 0:768], in_=wfd[:, 0:768]).then_inc(dsemA, 16)
            sync.dma_start(out=wfsb[:, 768:FB2], in_=wfd[:, 768:FB2]).then_inc(
                dsemA2, 16)
            sync.dma_start(
                out=wfsb[:, FB2:WB2], in_=wfd[:, FB2:WB2]
            ).then_inc(dsemBF, 16)
            sync.wait_ge(vsem, 5)
            sync.sem_clear(vsem)
            sync.dma_start(out=outd[:], in_=prob[:]).then_inc(osem, 16)
            sync.wait_ge(osem, 16)
            sync.sem_clear(osem)

        @block.tensor
        def _(tensor):
            tensor.wait_ge(dsemA, 16)
            tensor.sem_clear(dsemA)
            waited_a2 = [False]
            waited_bw = [smax <= 1]
            waited_bf = [False]
            pe_war = [0]
            for rec in sched:
                for (bank, w, grp, slot, war_need) in rec["tiles"]:
                    if slot >= 1 and not waited_bw[0]:
                        tensor.wait_ge(dsemBW, 16)
                        tensor.sem_clear(dsemBW)
                        waited_bw[0] = True
                    if rec["t"] >= 1 and not waited_bf[0]:
                        tensor.wait_ge(dsemBF, 16)
                        tensor.sem_clear(dsemBF)
                        waited_bf[0] = True
                    if war_need > pe_war[0]:
                        tensor.wait_ge(asem, war_need)
                        pe_war[0] = war_need
                    t = rec["t"]
                    ps = psall[:, bank * TK : bank * TK + w]
                    rows = slice(32 * grp, 32 * grp + 16)
                    tp = (32 * grp, 0)
                    c0, _ = wh_cols(slot, w)
                    wh = wfsb[rows, c0 : c0 + w]
                    c0, _ = wl_cols(slot, w)
                    wl = wfsb[rows, c0 : c0 + w]
                    c0, _ = fh_cols(t)
                    fh = wfsb[rows, c0 : c0 + 128]
                    c0, _ = fl_cols(t)
                    fl = wfsb[rows, c0 : c0 + 128]
                    tensor.matmul(ps, lhsT=fh, rhs=wh,
                                  start=True, stop=False, tile_position=tp)
                    if not waited_a2[0]:
                        tensor.wait_ge(dsemA2, 16)
                        tensor.sem_clear(dsemA2)
                        waited_a2[0] = True
                    tensor.matmul(ps, lhsT=fh, rhs=wl,
                                  start=False, stop=False, tile_position=tp)
                    tensor.matmul(ps, lhsT=fl, rhs=wh,
                                  start=False, stop=True, tile_position=tp).then_inc(
                        psem
                    )

        @block.scalar
        def _(scalar):
            # warm the Exp spline tables while DMAs are in flight (scale=0
            # reads nothing: exp(0*x+0)=1)
            scalar.activation(dummy, dummy, Exp, scale=0.0)
            if smax > 1:
                # set B-W: W slot1+ on the ACT HWDGE ring, in parallel
                scalar.dma_start(
                    out=wfsb[:, WB2:X], in_=wfd[:, WB2:X]
                ).then_inc(dsemBW, 16)
            scalar.wait_ge(vsem, 1)
            for rec in sched:
                scalar.wait_ge(psem, rec["psem_need"])
                pchunk = psall[:, rec["b0"] * TK : rec["b0"] * TK + rec["fd"]]
                sc = scratch[:, rec["scr_off"] : rec["scr_off"] + rec["fd"]]
                which, col = rec["acc"]
                dst = (accp if which == "p" else accn)[:, col : col + 1]
                scalar.activation(sc, pchunk, Exp, accum_out=dst).then_inc(asem)
            scalar.wait_ge(vsem, 3)
            scalar.sem_clear(psem)
            scalar.activation(z, s_, Exp, scale=-1.0).then_inc(asem)

        @block.vector
        def _(vector):
            vector.memset(accp[:], 0.0)
            vector.memset(accn[:], 0.0).then_inc(vsem)
            vector.wait_ge(asem, NCH)
            vector.reduce_sum(
                spos,
                accp[:].rearrange("p (t c) -> p t c", c=ncp),
                axis=mybir.AxisListType.X,
            )
            if ncn:
                vector.reduce_sum(
                    sneg,
                    accn[:].rearrange("p (t c) -> p t c", c=ncn),
                    axis=mybir.AxisListType.X,
                ).then_inc(vsem)
            else:
                vector.memset(sneg, 0.0).then_inc(vsem)
            # same-engine RAW edges still get sem'd (engine pipelines)
            vector.wait_ge(vsem, 2)
            vector.tensor_sub(s_, spos, sneg).then_inc(vsem)
            vector.wait_ge(asem, NCH + 1)
            vector.sem_clear(asem)
            vector.tensor_scalar_add(zp, z, 1.0).then_inc(vsem)
            vector.wait_ge(vsem, 4)
            vector.reciprocal(prob, zp).then_inc(vsem)

    return nc


def _build_graph(key):
    nkt_pos, nkt_neg, npos, nneg = key
    import concourse.bass as bass
    import concourse.mybir as mybir
    from concourse.tile import TileContext

    f32 = mybir.dt.float32
    f32r = mybir.dt.float32r
    Exp = mybir.ActivationFunctionType.Exp
    scratch_dt = {
        "f8": mybir.dt.float8e4, "bf16": mybir.dt.bfloat16, "f32": f32
    }[SCRATCH_DT]

    NKT = nkt_pos + nkt_neg
    KTOT = NKT * TK
    per_t = [_tiles_and_chunks(nkt_pos, nkt_neg, npos, nneg, t) for t in range(NT)]
    ncp = max(sum(1 for c in ch if c[2] == 0) for _, ch in per_t)
    ncn = max(sum(1 for c in ch if c[2] == 1) for _, ch in per_t)

    nc = bass.Bass()
    smax = (NKT + NGRP - 1) // NGRP
    X = 2 * smax * TK + 2 * NLOC
    # row-group layout: NGRP blocks of 16 partitions at base 32g, each
    # holding its quarter of the k-tiles (hi|lo) plus an F copy (hi|lo)
    wfd = nc.declare_dram_parameter("wf", [128, X], f32r, isOutput=False)
    outd = nc.declare_dram_parameter("out", [128, NT], f32, isOutput=True)
    fbase = 2 * smax * TK

    with TileContext(nc) as tc:
        with (
            tc.tile_pool(name="const", bufs=1) as cpool,
            tc.tile_pool(name="psum", bufs=1, space="PSUM") as ppool,
        ):
            wfsb = cpool.tile([128, X], f32r)
            # accumulator dumps: one fp32 per (n-tile, chunk), padded to a
            # 32-byte stride so no two writes share a cacheline (shared lines
            # create same-engine WAW deps -> extra sync waits -> walrus error)
            accp = cpool.tile([128, NT * ncp * 8], f32)
            if ncn:
                accn = cpool.tile([128, NT * ncn * 8], f32, tag="accn")
            else:
                accn = None
            # Exp writes its (unused) elementwise output here: a distinct
            # region per activation, never reused and never read, so every
            # activation's only dependency is the PE matmul semaphore (the
            # per-instruction sync-wait table only fits one wait)
            scratch = cpool.tile([128, NT * KTOT], scratch_dt)
            # one persistent tile spanning all 8 PSUM banks, rotated manually:
            # pool-slot rotation would add same-engine release waits that
            # overflow the 1-deep per-instruction sync-wait table
            psall = ppool.tile([128, 8 * TK], f32)

            # F first (first thing every matmul needs), then the W k-tiles
            nc.sync.dma_start(out=wfsb[:, fbase : fbase + 2 * NLOC],
                              in_=wfd[:, fbase : fbase + 2 * NLOC])
            nc.sync.dma_start(out=wfsb[:, 0:TK], in_=wfd[:, 0:TK])
            nc.sync.dma_start(out=wfsb[:, smax * TK : (smax + 1) * TK],
                              in_=wfd[:, smax * TK : (smax + 1) * TK])
            if smax > 1:
                nc.sync.dma_start(out=wfsb[:, TK : smax * TK],
                                  in_=wfd[:, TK : smax * TK])
                nc.sync.dma_start(
                    out=wfsb[:, (smax + 1) * TK : 2 * smax * TK],
                    in_=wfd[:, (smax + 1) * TK : 2 * smax * TK])

            # some n-tiles have fewer chunks than ncp/ncn; zero the unwritten
            # accumulator columns once
            nc.vector.memset(accp[:], 0.0)
            if ncn:
                nc.vector.memset(accn[:], 0.0)

            for t in range(NT):
                tiles, chunks = per_t[t]
                cpt = cnt_ = 0
                for (j0, ln, g, fd) in chunks:
                    for j in range(j0, j0 + ln):
                        w = tiles[j][2]
                        bank = (t * NKT + j) % 8
                        ps = psall[:, bank * TK : bank * TK + w]
                        grp, slot = j % NGRP, j // NGRP
                        rows = slice(32 * grp, 32 * grp + 16)
                        tp = (32 * grp, 0)
                        wh = wfsb[rows, slot * TK : slot * TK + w]
                        wl = wfsb[rows, (smax + slot) * TK : (smax + slot) * TK + w]
                        fh = wfsb[rows, fbase + t * 128 : fbase + (t + 1) * 128]
                        fl = wfsb[
                            rows,
                            fbase + NLOC + t * 128 : fbase + NLOC + (t + 1) * 128,
                        ]
                        nc.tensor.matmul(
                            ps, lhsT=fh, rhs=wh,
                            start=True, stop=False, tile_position=tp,
                        )
                        nc.tensor.matmul(
                            ps, lhsT=fh, rhs=wl,
                            start=False, stop=False, tile_position=tp,
                        )
                        nc.tensor.matmul(
                            ps, lhsT=fl, rhs=wh,
                            start=False, stop=True, tile_position=tp,
                        )
                    # one Exp over the whole chunk; only the fused accumulator
                    # dump is consumed
                    b0 = (t * NKT + j0) % 8
                    pchunk = psall[:, b0 * TK : b0 * TK + fd]
                    sc = scratch[:, t * KTOT + j0 * TK : t * KTOT + j0 * TK + fd]
                    if g == 0:
                        col = (t * ncp + cpt) * 8
                        cpt += 1
                        dst = accp[:, col : col + 1]
                    else:
                        col = (t * ncn + cnt_) * 8
                        cnt_ += 1
                        dst = accn[:, col : col + 1]
                    nc.scalar.activation(sc, pchunk, Exp, accum_out=dst)

            # epilogue: S = sum(pos) - sum(neg); prob = 1/(1+exp(-S))
            spos = cpool.tile([128, NT], f32)
            sneg = cpool.tile([128, NT], f32)
            accp_v = accp[:].rearrange("p (t c e) -> p t c e", c=ncp, e=8)[:, :, :, 0:1]
            nc.vector.reduce_sum(spos[:], accp_v, axis=mybir.AxisListType.XY)
            if ncn:
                accn_v = accn[:].rearrange(
                    "p (t c e) -> p t c e", c=ncn, e=8
                )[:, :, :, 0:1]
                nc.vector.reduce_sum(sneg[:], accn_v, axis=mybir.AxisListType.XY)
            else:
                nc.vector.memset(sneg[:], 0.0)
            s = cpool.tile([128, NT], f32)
            nc.vector.tensor_sub(s[:], spos[:], sneg[:])
            # sigmoid(s) = 1 / (1 + exp(-s)); Exp table is already loaded
            z = cpool.tile([128, NT], f32)
            nc.scalar.activation(z[:], s[:], Exp, scale=-1.0)
            zp = cpool.tile([128, NT], f32)
            nc.vector.tensor_scalar_add(zp[:], z[:], 1.0)
            prob = cpool.tile([128, NT], f32)
            nc.vector.reciprocal(prob[:], zp[:])
            nc.sync.dma_start(out=outd[:], in_=prob[:])

    _legalize_waits(nc, mybir)
    return nc


def _legalize_waits(nc, mybir):
    """The TRN2 per-instruction sync-wait table is effectively one entry for
    datapath instructions; hoist excess semaphore waits onto same-engine NOPs
    inserted immediately before (program order on the same queue preserves
    semantics)."""
    cnt = [0]
    for fn in nc.m.functions:
        for bb in fn.blocks:
            new = []
            for ins in bb.instructions:
                si = ins.sync_info
                if si is not None and si.on_wait and len(si.on_wait) > 1:
                    waits = list(si.on_wait)
                    for w in waits[:-1]:
                        cnt[0] += 1
                        nop = mybir.InstNoOp(
                            name=f"I-waitfix-{cnt[0]}",
                            engine=ins.engine,
                            sync_info=mybir.SyncInfo(on_wait=[w], on_update=[]),
                        )
                        new.append(nop)
                    si.on_wait = [waits[-1]]
                new.append(ins)
            bb.instructions = new


def _ensure_ntff_hook():
    """Shim: this image's antenv lacks axon_hooks; inject it and register the
    ctypes NTFF profile hook so trace=True can measure HW exec time."""
    try:
        from antenv.axon_hooks import get_axon_ntff_profile_hook  # noqa: F401
        return
    except ImportError:
        pass
    import types

    import antenv

    mod = types.ModuleType("antenv.axon_hooks")
    mod._hook = None

    def set_axon_ntff_profile_hook(h):
        mod._hook = h

    def get_axon_ntff_profile_hook():
        return mod._hook

    mod.set_axon_ntff_profile_hook = set_axon_ntff_profile_hook
    mod.get_axon_ntff_profile_hook = get_axon_ntff_profile_hook
    sys.modules["antenv.axon_hooks"] = mod
    antenv.axon_hooks = mod
    try:
        from trn_agent_boot.trn_boot import _ntff_profile_via_ctypes

        hook = _ntff_profile_via_ctypes("/opt/axon/libaxon_pjrt.so")
        if hook is not None:
            mod._hook = hook
    except Exception:
        pass


def _make_in_maps(W, F, nkt_pos, nkt_neg):
    Whi = _round_f32r(W)
    Wlo = _round_f32r(W - Whi)
    Fhi = _round_f32r(F)
    Flo = _round_f32r(F - Fhi)
    NKT = nkt_pos + nkt_neg
    smax = (NKT + NGRP - 1) // NGRP
    FB2 = 3 * TK
    WB2 = FB2 + 2 * (NLOC - 128)
    X = WB2 + 2 * (smax - 1) * TK
    base = np.zeros((128, X), dtype=np.float32)
    for g in range(NGRP):
        rows = slice(32 * g, 32 * g + 16)
        js = [j for j in range(NKT) if j % NGRP == g]
        for s, j in enumerate(js):
            wh = Whi[:, j * TK : (j + 1) * TK]
            wl = Wlo[:, j * TK : (j + 1) * TK]
            if s == 0:
                base[rows, 256 : 256 + TK] = wh
                base[rows, 256 + TK : 256 + 2 * TK] = wl
            else:
                base[rows, WB2 + (s - 1) * TK : WB2 + s * TK] = wh
                base[rows,
                     WB2 + (smax - 1 + s - 1) * TK : WB2 + (smax + s - 1) * TK] = wl
    in_maps = []
    for c in range(NCORES):
        cs = c * NLOC
        buf = base.copy()
        for g in range(NGRP):
            rows = slice(32 * g, 32 * g + 16)
            buf[rows, 0:128] = Fhi[:, cs : cs + 128]
            buf[rows, 128:256] = Flo[:, cs : cs + 128]
            buf[rows, FB2 : FB2 + NLOC - 128] = Fhi[:, cs + 128 : cs + NLOC]
            buf[rows, FB2 + NLOC - 128 : WB2] = Flo[:, cs + 128 : cs + NLOC]
        in_maps.append({"wf": buf})
    return in_maps


def kernel(origins, directions, embeddings, chol, labels, idx):
    global LAST_EXEC_TIME_NS
    import concourse.bass_utils as bass_utils
    from concourse.bass_utils import run_bass_kernel_spmd

    W, F, nkt_pos, nkt_neg, npos, nneg = _host_prep(
        origins, directions, embeddings, chol, labels, idx
    )

    raw = os.environ.get("KERNEL_RAW", "1") == "1"
    key = (nkt_pos, nkt_neg, npos, nneg, raw)
    if key not in _GRAPH_CACHE:
        _GRAPH_CACHE[key] = (
            _build_graph_raw(key[:4]) if raw else _build_graph(key[:4])
        )
    nc = _GRAPH_CACHE[key]

    in_maps = _make_in_maps(W, F, nkt_pos, nkt_neg)

    trace = os.environ.get("KERNEL_TRACE", "0") == "1"
    if trace:
        _ensure_ntff_hook()
        bass_utils.upload_artifacts = lambda tmpdir: tmpdir  # no bucket in container
    res = run_bass_kernel_spmd(nc, in_maps, core_ids=list(range(NCORES)), trace=trace)
    LAST_EXEC_TIME_NS = res.exec_time_ns

    out = np.empty((N,), dtype=np.float32)
    for c in range(NCORES):
        oc = res.results[c]["out"]                # [128, NT], out[p, t] = ray t*128+p
        out[c * NLOC : (c + 1) * NLOC] = np.asarray(oc).T.reshape(-1)
    return out.reshape(-1, 1)
